# revision 1
# baseline (speedup 1.0000x reference)
"""Trainium2 Bass kernel for PersonalizedSimpleAttention.

Computation (per batch b, person p = person_idxs[b]):
    k0 = x @ Wk.T ; q0 = x @ Wq.T ; v = x @ Wv.T          # [T, KH]
    keys    = k0 @ PK_W[p].T + PK_b[p]
    queries = (q0 @ PQ_W[p].T + PQ_b[p]) / sqrt(KH)
    attn    = softmax(queries @ keys.T + maskbias, axis=-1)
    out     = attn @ v                                     # [T, VH]

Sharding: data-parallel over batch across 8 cores (8 batches each).  The
per-person weights are gathered on the host (pure indexing) so each core
receives exactly its 8 [KH, KH] weight matrices.  All on-device layouts are
transposed ([feature, token]) so every matmul contracts over the partition
dim with no on-device transposes; softmax runs over the partition (key) dim
using a strided free-dim reduce + gpsimd partition_all_reduce, and the
normalization is folded in after the attn@v matmul (divide by denom once on
[KH, T] instead of on [T, T]).

Matmuls run in fp32r (TF32-like, 1 cycle/row vs fp32's 4) with fp32 PSUM
accumulation.
"""
import math

import numpy as np

import concourse.bass as bass  # noqa: F401  (registers engines)
import concourse.mybir as mybir
from concourse import bacc
from concourse.bass_utils import run_bass_kernel_spmd
from concourse.tile import TileContext

F32 = mybir.dt.float32
F32R = mybir.dt.float32r
AF = mybir.ActivationFunctionType

B, T, EMB, KH = 64, 1024, 128, 256
NCORES = 8
BPC = B // NCORES          # batches per core
ST = T // 128              # 8 key tiles of 128
TB = T // 512              # 2 moving-dim blocks of 512

_CACHE = {}


def _build_nc():
    nc = bacc.Bacc("TRN2", target_bir_lowering=False, debug=False)

    xT = nc.declare_dram_parameter("xT", [BPC, EMB, T], F32R, isOutput=False)
    wk = nc.declare_dram_parameter("wkT", [EMB, KH], F32R, isOutput=False)
    wq = nc.declare_dram_parameter("wqT", [EMB, KH], F32R, isOutput=False)
    wv = nc.declare_dram_parameter("wvT", [EMB, KH], F32R, isOutput=False)
    pkw = nc.declare_dram_parameter("pkwT", [BPC, KH, KH], F32R, isOutput=False)
    pqw = nc.declare_dram_parameter("pqwT", [BPC, KH, KH], F32R, isOutput=False)
    pkb = nc.declare_dram_parameter("pkb", [BPC, KH], F32, isOutput=False)
    pqb = nc.declare_dram_parameter("pqb", [BPC, KH], F32, isOutput=False)
    mb = nc.declare_dram_parameter("mb", [BPC, T], F32, isOutput=False)
    out = nc.declare_dram_parameter("out", [BPC, KH, T], F32, isOutput=True)

    with TileContext(nc) as tc:
        with tc.tile_pool(name="const", bufs=1) as cpool, \
             tc.tile_pool(name="work", bufs=2) as wpool, \
             tc.tile_pool(name="big", bufs=1) as bpool, \
             tc.tile_pool(name="psa", bufs=3, space="PSUM") as psa, \
             tc.tile_pool(name="psb", bufs=2, space="PSUM") as psb:

            wkt = cpool.tile([128, KH], F32R, name="wkt")
            wqt = cpool.tile([128, KH], F32R, name="wqt")
            wvt = cpool.tile([128, KH], F32R, name="wvt")
            nc.sync.dma_start(out=wkt, in_=wk[:, :])
            nc.sync.dma_start(out=wqt, in_=wq[:, :])
            nc.sync.dma_start(out=wvt, in_=wv[:, :])

            for b in range(BPC):
                # ---- load per-batch operands -------------------------------
                xt = wpool.tile([128, T], F32R, name="xt")
                nc.sync.dma_start(out=xt, in_=xT[b])

                # PK_W[p].T is [h, o]; rows h0/h1 side by side on free axis
                pkwt = wpool.tile([128, 2 * KH], F32R, name="pkwt")
                pqwt = wpool.tile([128, 2 * KH], F32R, name="pqwt")
                for hh in range(2):
                    nc.sync.dma_start(out=pkwt[:, hh * KH:(hh + 1) * KH],
                                      in_=pkw[b, hh * 128:(hh + 1) * 128, :])
                    nc.sync.dma_start(out=pqwt[:, hh * KH:(hh + 1) * KH],
                                      in_=pqw[b, hh * 128:(hh + 1) * 128, :])
                pkbt = wpool.tile([128, 2], F32, name="pkbt")
                pqbt = wpool.tile([128, 2], F32, name="pqbt")
                mbt = wpool.tile([128, ST], F32, name="mbt")
                nc.sync.dma_start(out=pkbt, in_=pkb[b].rearrange("(a p) -> p a", p=128))
                nc.sync.dma_start(out=pqbt, in_=pqb[b].rearrange("(a p) -> p a", p=128))
                nc.sync.dma_start(out=mbt, in_=mb[b].rearrange("(a p) -> p a", p=128))

                # ---- shared projections -----------------------------------
                # k0T/q0T: [h, s] as [128, hh*T + s];  v: [s, d] as [128, st*KH + d]
                k0t = wpool.tile([128, 2 * T], F32R, name="k0t")
                q0t = wpool.tile([128, 2 * T], F32R, name="q0t")
                vt = wpool.tile([128, ST * KH], F32R, name="vt")
                for hh in range(2):
                    for sb in range(TB):
                        psk = psa.tile([128, 512], F32, name="psk", tag="a")
                        nc.tensor.matmul(psk, wkt[:, hh * 128:(hh + 1) * 128],
                                         xt[:, sb * 512:(sb + 1) * 512])
                        nc.scalar.copy(k0t[:, hh * T + sb * 512:hh * T + (sb + 1) * 512], psk)
                        psq = psa.tile([128, 512], F32, name="psq", tag="a")
                        nc.tensor.matmul(psq, wqt[:, hh * 128:(hh + 1) * 128],
                                         xt[:, sb * 512:(sb + 1) * 512])
                        nc.vector.tensor_copy(q0t[:, hh * T + sb * 512:hh * T + (sb + 1) * 512], psq)
                for st in range(ST):
                    psv = psa.tile([128, KH], F32, name="psv", tag="a")
                    nc.tensor.matmul(psv, xt[:, st * 128:(st + 1) * 128], wvt)
                    nc.scalar.copy(vt[:, st * KH:(st + 1) * KH], psv)

                # ---- personalized projections -----------------------------
                # keysT/queriesT: [o, s] as [128, oh*T + s]
                kt = wpool.tile([128, 2 * T], F32R, name="kt")
                qt = wpool.tile([128, 2 * T], F32R, name="qt")
                for (wt_, bt_, src, dst) in ((pkwt, pkbt, k0t, kt), (pqwt, pqbt, q0t, qt)):
                    for oh in range(2):
                        pp = [psa.tile([128, 512], F32, name=f"pp{sb}", tag="a")
                              for sb in range(TB)]
                        for hh in range(2):
                            lhs = wt_[:, hh * KH + oh * 128:hh * KH + (oh + 1) * 128]
                            for sb in range(TB):
                                nc.tensor.matmul(
                                    pp[sb], lhs,
                                    src[:, hh * T + sb * 512:hh * T + (sb + 1) * 512],
                                    start=(hh == 0), stop=(hh == 1))
                        for sb in range(TB):
                            nc.vector.tensor_scalar_add(
                                dst[:, oh * T + sb * 512:oh * T + (sb + 1) * 512],
                                pp[sb], bt_[:, oh:oh + 1])

                # ---- attention scores + exp -------------------------------
                # dotT: [s, t]; E[s, t] = exp(dot + maskbias[s])
                et = bpool.tile([128, ST * T], F32R, name="et", bufs=1)
                for st in range(ST):
                    psd = psb.tile([128, T], F32, name="psd", tag="b")
                    for dh in range(2):
                        lhs = kt[:, dh * T + st * 128:dh * T + (st + 1) * 128]
                        for tb in range(TB):
                            nc.tensor.matmul(
                                psd[:, tb * 512:(tb + 1) * 512], lhs,
                                qt[:, dh * T + tb * 512:dh * T + (tb + 1) * 512],
                                start=(dh == 0), stop=(dh == 1))
                    nc.scalar.activation(et[:, st * T:(st + 1) * T], psd,
                                         AF.Exp, bias=mbt[:, st:st + 1])

                # ---- softmax denominator ----------------------------------
                # etot[p, t] = sum_st E[st*128+p, t]  (strided free-dim reduce)
                # denbc[q, t] = sum_p etot[p, t]      (gpsimd partition all-reduce)
                etot = bpool.tile([128, T], F32, name="etot", bufs=1)
                nc.vector.reduce_sum(
                    etot,
                    et.bitcast(F32).rearrange("p (st t) -> p t st", st=ST),
                    axis=mybir.AxisListType.X)
                denbc = bpool.tile([128, T], F32, name="denbc", bufs=1)
                nc.gpsimd.partition_all_reduce(
                    denbc, etot, channels=128, reduce_op=bass.bass_isa.ReduceOp.add)
                rcp = bpool.tile([128, T], F32, name="rcp", bufs=1)
                nc.vector.reciprocal(rcp, denbc)

                # ---- context: ctxT[d, t] = sum_s v[s, d] * E[s, t] --------
                ctx = wpool.tile([128, 2 * T], F32, name="ctx")
                for dh in range(2):
                    psc = psb.tile([128, T], F32, name="psc", tag="b")
                    for st in range(ST):
                        lhs = vt[:, st * KH + dh * 128:st * KH + (dh + 1) * 128]
                        for tb in range(TB):
                            nc.tensor.matmul(
                                psc[:, tb * 512:(tb + 1) * 512], lhs,
                                et[:, st * T + tb * 512:st * T + (tb + 1) * 512],
                                start=(st == 0), stop=(st == ST - 1))
                    nc.vector.tensor_mul(ctx[:, dh * T:(dh + 1) * T], psc, rcp)
                    nc.sync.dma_start(out=out[b, dh * 128:(dh + 1) * 128, :],
                                      in_=ctx[:, dh * T:(dh + 1) * T])

    nc.compile()
    return nc


def _get_nc():
    if "nc" not in _CACHE:
        _CACHE["nc"] = _build_nc()
    return _CACHE["nc"]


def kernel(x, mask, person_idxs, Wk, Wq, Wv, PK_W, PK_b, PQ_W, PQ_b):
    x = np.asarray(x, dtype=np.float32)
    mask = np.asarray(mask)
    idx = np.asarray(person_idxs).astype(np.int64)
    sk = 1.0 / math.sqrt(KH)

    wkT = np.ascontiguousarray(np.asarray(Wk, np.float32).T)      # [EMB, KH]
    wqT = np.ascontiguousarray(np.asarray(Wq, np.float32).T)
    wvT = np.ascontiguousarray(np.asarray(Wv, np.float32).T)
    mbias = np.where(mask[:, 0, :], 0.0, -30.0).astype(np.float32)  # [B, T]

    in_maps = []
    for c in range(NCORES):
        bs = slice(c * BPC, (c + 1) * BPC)
        ci = idx[bs]
        in_maps.append({
            "xT": np.ascontiguousarray(x[bs].transpose(0, 2, 1)),          # [BPC, EMB, T]
            "wkT": wkT, "wqT": wqT, "wvT": wvT,
            "pkwT": np.ascontiguousarray(np.asarray(PK_W, np.float32)[ci].transpose(0, 2, 1)),
            "pqwT": np.ascontiguousarray((np.asarray(PQ_W, np.float32)[ci] * sk).transpose(0, 2, 1)),
            "pkb": np.ascontiguousarray(np.asarray(PK_b, np.float32)[ci]),
            "pqb": np.ascontiguousarray(np.asarray(PQ_b, np.float32)[ci] * sk),
            "mb": np.ascontiguousarray(mbias[bs]),
        })

    nc = _get_nc()
    res = run_bass_kernel_spmd(nc, in_maps, list(range(NCORES)))
    outT = np.concatenate([res.results[c]["out"] for c in range(NCORES)], axis=0)
    return np.ascontiguousarray(outT.transpose(0, 2, 1))          # [B, T, KH]


# revision 2
# speedup vs baseline: 1.1455x; 1.1455x over previous
"""Trainium2 Bass kernel for PersonalizedSimpleAttention.

Computation (per batch b, person p = person_idxs[b]):
    k0 = x @ Wk.T ; q0 = x @ Wq.T ; v = x @ Wv.T          # [T, KH]
    keys    = k0 @ PK_W[p].T + PK_b[p]
    queries = (q0 @ PQ_W[p].T + PQ_b[p]) / sqrt(KH)
    attn    = softmax(queries @ keys.T + maskbias, axis=-1)
    out     = attn @ v                                     # [T, VH]

Sharding: data-parallel over batch across 8 cores (8 batches each).  The
per-person weights are gathered on the host (pure indexing) so each core
receives exactly its 8 [KH, KH] weight matrices.  All on-device layouts are
transposed ([feature, token]) so every matmul contracts over the partition
dim with no on-device transposes; softmax runs over the partition (key) dim
using a strided free-dim reduce + gpsimd partition_all_reduce, and the
normalization is folded in after the attn@v matmul (divide by denom once on
[KH, T] instead of on [T, T]).

Matmul operand dtype is selectable (bf16 default; f32r = TF32-like; f32)
with fp32 PSUM accumulation throughout.
"""
import math
import os

import numpy as np

import concourse.bass as bass  # noqa: F401  (registers engines)
import concourse.mybir as mybir
from concourse import bacc
from concourse.bass_utils import run_bass_kernel_spmd
from concourse.tile import TileContext

F32 = mybir.dt.float32
AF = mybir.ActivationFunctionType

B, T, EMB, KH = 64, 1024, 128, 256
NCORES = 8
BPC = B // NCORES          # batches per core
ST = T // 128              # 8 key tiles of 128
TB = T // 512              # 2 moving-dim blocks of 512

DT_MM_NAME = os.environ.get("BASS_KERNEL_DT", "bf16")
_DT_MAP = {"bf16": mybir.dt.bfloat16, "f32r": mybir.dt.float32r, "f32": F32}

_CACHE = {}


def _build_nc(dt_mm):
    nc = bacc.Bacc("TRN2", target_bir_lowering=False, debug=False)

    xT = nc.declare_dram_parameter("xT", [BPC, EMB, T], dt_mm, isOutput=False)
    wk = nc.declare_dram_parameter("wkT", [EMB, KH], dt_mm, isOutput=False)
    wq = nc.declare_dram_parameter("wqT", [EMB, KH], dt_mm, isOutput=False)
    wv = nc.declare_dram_parameter("wvT", [EMB, KH], dt_mm, isOutput=False)
    pkw = nc.declare_dram_parameter("pkwT", [BPC, KH, KH], dt_mm, isOutput=False)
    pqw = nc.declare_dram_parameter("pqwT", [BPC, KH, KH], dt_mm, isOutput=False)
    pkb = nc.declare_dram_parameter("pkb", [BPC, KH], F32, isOutput=False)
    pqb = nc.declare_dram_parameter("pqb", [BPC, KH], F32, isOutput=False)
    mb = nc.declare_dram_parameter("mb", [BPC, T], F32, isOutput=False)
    out = nc.declare_dram_parameter("out", [BPC, KH, T], F32, isOutput=True)

    with TileContext(nc) as tc:
        with tc.tile_pool(name="const", bufs=1) as cpool, \
             tc.tile_pool(name="work", bufs=2) as wpool, \
             tc.tile_pool(name="big", bufs=1) as bpool, \
             tc.tile_pool(name="psa", bufs=3, space="PSUM") as psa, \
             tc.tile_pool(name="psb", bufs=2, space="PSUM") as psb:

            wkt = cpool.tile([128, KH], dt_mm, name="wkt")
            wqt = cpool.tile([128, KH], dt_mm, name="wqt")
            wvt = cpool.tile([128, KH], dt_mm, name="wvt")
            nc.sync.dma_start(out=wkt, in_=wk[:, :])
            nc.sync.dma_start(out=wqt, in_=wq[:, :])
            nc.sync.dma_start(out=wvt, in_=wv[:, :])

            for b in range(BPC):
                # ---- load per-batch operands -------------------------------
                xt = wpool.tile([128, T], dt_mm, name="xt")
                nc.sync.dma_start(out=xt, in_=xT[b])

                # PK_W[p].T is [h, o]; rows h0/h1 side by side on free axis
                pkwt = wpool.tile([128, 2 * KH], dt_mm, name="pkwt")
                pqwt = wpool.tile([128, 2 * KH], dt_mm, name="pqwt")
                for hh in range(2):
                    nc.sync.dma_start(out=pkwt[:, hh * KH:(hh + 1) * KH],
                                      in_=pkw[b, hh * 128:(hh + 1) * 128, :])
                    nc.sync.dma_start(out=pqwt[:, hh * KH:(hh + 1) * KH],
                                      in_=pqw[b, hh * 128:(hh + 1) * 128, :])
                pkbt = wpool.tile([128, 2], F32, name="pkbt")
                pqbt = wpool.tile([128, 2], F32, name="pqbt")
                mbt = wpool.tile([128, ST], F32, name="mbt")
                nc.sync.dma_start(out=pkbt, in_=pkb[b].rearrange("(a p) -> p a", p=128))
                nc.sync.dma_start(out=pqbt, in_=pqb[b].rearrange("(a p) -> p a", p=128))
                nc.sync.dma_start(out=mbt, in_=mb[b].rearrange("(a p) -> p a", p=128))

                # ---- shared projections -----------------------------------
                # k0T/q0T: [h, s] as [128, hh*T + s];  v: [s, d] as [128, st*KH + d]
                k0t = wpool.tile([128, 2 * T], dt_mm, name="k0t")
                q0t = wpool.tile([128, 2 * T], dt_mm, name="q0t")
                vt = wpool.tile([128, ST * KH], dt_mm, name="vt")
                for hh in range(2):
                    for sb in range(TB):
                        psk = psa.tile([128, 512], F32, name="psk", tag="a")
                        nc.tensor.matmul(psk, wkt[:, hh * 128:(hh + 1) * 128],
                                         xt[:, sb * 512:(sb + 1) * 512])
                        nc.scalar.copy(k0t[:, hh * T + sb * 512:hh * T + (sb + 1) * 512], psk)
                        psq = psa.tile([128, 512], F32, name="psq", tag="a")
                        nc.tensor.matmul(psq, wqt[:, hh * 128:(hh + 1) * 128],
                                         xt[:, sb * 512:(sb + 1) * 512])
                        nc.vector.tensor_copy(q0t[:, hh * T + sb * 512:hh * T + (sb + 1) * 512], psq)
                for st in range(ST):
                    psv = psa.tile([128, KH], F32, name="psv", tag="a")
                    nc.tensor.matmul(psv, xt[:, st * 128:(st + 1) * 128], wvt)
                    nc.scalar.copy(vt[:, st * KH:(st + 1) * KH], psv)

                # ---- personalized projections -----------------------------
                # keysT/queriesT: [o, s] as [128, oh*T + s]
                kt = wpool.tile([128, 2 * T], dt_mm, name="kt")
                qt = wpool.tile([128, 2 * T], dt_mm, name="qt")
                for (wt_, bt_, src, dst) in ((pkwt, pkbt, k0t, kt), (pqwt, pqbt, q0t, qt)):
                    for oh in range(2):
                        pp = [psa.tile([128, 512], F32, name=f"pp{sb}", tag="a")
                              for sb in range(TB)]
                        for hh in range(2):
                            lhs = wt_[:, hh * KH + oh * 128:hh * KH + (oh + 1) * 128]
                            for sb in range(TB):
                                nc.tensor.matmul(
                                    pp[sb], lhs,
                                    src[:, hh * T + sb * 512:hh * T + (sb + 1) * 512],
                                    start=(hh == 0), stop=(hh == 1))
                        for sb in range(TB):
                            nc.vector.tensor_scalar_add(
                                dst[:, oh * T + sb * 512:oh * T + (sb + 1) * 512],
                                pp[sb], bt_[:, oh:oh + 1])

                # ---- attention scores + exp -------------------------------
                # dotT: [s, t]; E[s, t] = exp(dot + maskbias[s])
                et = bpool.tile([128, ST * T], dt_mm, name="et", bufs=1)
                for st in range(ST):
                    psd = psb.tile([128, T], F32, name="psd", tag="b")
                    for dh in range(2):
                        lhs = kt[:, dh * T + st * 128:dh * T + (st + 1) * 128]
                        for tb in range(TB):
                            nc.tensor.matmul(
                                psd[:, tb * 512:(tb + 1) * 512], lhs,
                                qt[:, dh * T + tb * 512:dh * T + (tb + 1) * 512],
                                start=(dh == 0), stop=(dh == 1))
                    nc.scalar.activation(et[:, st * T:(st + 1) * T], psd,
                                         AF.Exp, bias=mbt[:, st:st + 1])

                # ---- softmax denominator ----------------------------------
                # etot[p, t] = sum_st E[st*128+p, t]  (strided free-dim reduce)
                # denbc[q, t] = sum_p etot[p, t]      (gpsimd partition all-reduce)
                etot = bpool.tile([128, T], F32, name="etot", bufs=1)
                nc.vector.reduce_sum(
                    etot,
                    et.rearrange("p (st t) -> p t st", st=ST),
                    axis=mybir.AxisListType.X)
                denbc = bpool.tile([128, T], F32, name="denbc", bufs=1)
                nc.gpsimd.partition_all_reduce(
                    denbc, etot, channels=128, reduce_op=bass.bass_isa.ReduceOp.add)
                rcp = bpool.tile([128, T], F32, name="rcp", bufs=1)
                nc.vector.reciprocal_approx_fast(rcp, denbc)

                # ---- context: ctxT[d, t] = sum_s v[s, d] * E[s, t] --------
                ctx = wpool.tile([128, 2 * T], F32, name="ctx")
                for dh in range(2):
                    psc = psb.tile([128, T], F32, name="psc", tag="b")
                    for st in range(ST):
                        lhs = vt[:, st * KH + dh * 128:st * KH + (dh + 1) * 128]
                        for tb in range(TB):
                            nc.tensor.matmul(
                                psc[:, tb * 512:(tb + 1) * 512], lhs,
                                et[:, st * T + tb * 512:st * T + (tb + 1) * 512],
                                start=(st == 0), stop=(st == ST - 1))
                    nc.vector.tensor_mul(ctx[:, dh * T:(dh + 1) * T], psc, rcp)
                    nc.sync.dma_start(out=out[b, dh * 128:(dh + 1) * 128, :],
                                      in_=ctx[:, dh * T:(dh + 1) * T])

    nc.compile()
    return nc


def _get_nc():
    if "nc" not in _CACHE:
        _CACHE["nc"] = _build_nc(_DT_MAP[DT_MM_NAME])
    return _CACHE["nc"]


def _np_mm_dtype():
    if DT_MM_NAME == "bf16":
        import ml_dtypes
        return np.dtype(ml_dtypes.bfloat16)
    return np.float32


def build_in_maps(x, mask, person_idxs, Wk, Wq, Wv, PK_W, PK_b, PQ_W, PQ_b):
    x = np.asarray(x, dtype=np.float32)
    mask = np.asarray(mask)
    idx = np.asarray(person_idxs).astype(np.int64)
    sk = 1.0 / math.sqrt(KH)
    mdt = _np_mm_dtype()

    wkT = np.ascontiguousarray(np.asarray(Wk, np.float32).T).astype(mdt)
    wqT = np.ascontiguousarray(np.asarray(Wq, np.float32).T).astype(mdt)
    wvT = np.ascontiguousarray(np.asarray(Wv, np.float32).T).astype(mdt)
    mbias = np.where(mask[:, 0, :], 0.0, -30.0).astype(np.float32)  # [B, T]

    in_maps = []
    for c in range(NCORES):
        bs = slice(c * BPC, (c + 1) * BPC)
        ci = idx[bs]
        in_maps.append({
            "xT": np.ascontiguousarray(x[bs].transpose(0, 2, 1)).astype(mdt),
            "wkT": wkT, "wqT": wqT, "wvT": wvT,
            "pkwT": np.ascontiguousarray(
                np.asarray(PK_W, np.float32)[ci].transpose(0, 2, 1)).astype(mdt),
            "pqwT": np.ascontiguousarray(
                (np.asarray(PQ_W, np.float32)[ci] * sk).transpose(0, 2, 1)).astype(mdt),
            "pkb": np.ascontiguousarray(np.asarray(PK_b, np.float32)[ci]),
            "pqb": np.ascontiguousarray(np.asarray(PQ_b, np.float32)[ci] * sk),
            "mb": np.ascontiguousarray(mbias[bs]),
        })
    return in_maps


def kernel(x, mask, person_idxs, Wk, Wq, Wv, PK_W, PK_b, PQ_W, PQ_b):
    in_maps = build_in_maps(x, mask, person_idxs, Wk, Wq, Wv, PK_W, PK_b, PQ_W, PQ_b)
    nc = _get_nc()
    res = run_bass_kernel_spmd(nc, in_maps, list(range(NCORES)))
    outT = np.concatenate([res.results[c]["out"] for c in range(NCORES)], axis=0)
    return np.ascontiguousarray(outT.transpose(0, 2, 1))          # [B, T, KH]
